# revision 20
# baseline (speedup 1.0000x reference)
"""CapsuleLayer (dynamic routing) Trainium2 kernel, 8-core SPMD.

Sharding: n_in (2048) split 8 ways -> 256 rows per core. W/x are sharded by n;
the only cross-core data is the [b, c, e] routing sum `s`, AllReduced once per
routing iteration (3x 256KB).

Device layout (per core):
  u[b, n, c, e] is produced by PE matmuls with K = (4n x 16d [+1 bias row]),
  M = 128 = (4 n-offsets x 32 batch), N = (e,c) chunks of 512. A partition row
  p of every on-chip [128, *] tensor is (j, b) = (p // 32, p % 32), i.e. n-local
  offset j within the 4-n group and batch b. The free axis of u is e-major
  (e*64 + c), which keeps every broadcast multiply at DVE 2x mode.

  Routing per iteration (streaming u from a DRAM fp16 scratch):
    db[p, c] = sum_e u * v   -> DVE mult + fp16 tree reduction over e
    softmax over c (free axis) -> ACT Exp with fused accum_out=Z, 1/Z on DVE
    s[b, (e,c)] += sum_n c * u -> DVE mult + PE matmul with a constant 0/1
      selector lhsT (sel[p, m] = p%32==m) contracting the partition axis per-b.
"""

import numpy as np
from contextlib import ExitStack

import concourse.bass as bass
import concourse.tile as tile
from concourse import mybir
from concourse.bass_utils import run_bass_kernel_spmd

F16 = mybir.dt.float16
F32 = mybir.dt.float32
AF = mybir.ActivationFunctionType
OP = mybir.AluOpType

N_CORES = 8
BT, NN, DD = 32, 2048, 16      # batch, n_in, d_in
CC, EE = 64, 32                # n_capsule, d_capsule
NL = NN // N_CORES             # 256 local n rows
G4 = 4                         # n rows per matmul group
NG = NL // G4                  # 64 groups
KK = G4 * DD + 1               # 65 contraction rows (incl. bias row)
CE = CC * EE                   # 2048, stored e-major: col = e*CC + c
EPS = 1e-9


def _split_waits(nc):
    """walrus CTRL codegen only supports one sem-wait per instruction; hoist
    extra waits into preceding NoOps on the same engine."""
    for f in nc.m.functions:
        for bb in f.blocks:
            new_insts = []
            for inst in bb.instructions:
                si = inst.sync_info
                if si is not None and si.on_wait and len(si.on_wait) > 1:
                    waits = list(si.on_wait)
                    for w in waits[:-1]:
                        new_insts.append(mybir.InstNoOp(
                            name=f"WS-{nc.next_id()}",
                            sync_info=mybir.SyncInfo(on_wait=[w], on_update=[]),
                            bass_nofuse=True,
                            engine=inst.engine,
                        ))
                    inst.sync_info = mybir.SyncInfo(
                        on_wait=waits[-1:], on_update=si.on_update)
                new_insts.append(inst)
            bb.instructions = new_insts


def _bcast(ap, n, axis_pos):
    """Insert a [step=0, count=n] dim into an AP at free-dim position axis_pos
    (0 = right after the partition dim)."""
    dims = [list(d) for d in ap.ap]
    dims.insert(1 + axis_pos, [0, n])
    return bass.AP(tensor=ap.tensor, offset=ap.offset, ap=dims)


def _build_program():
    nc = bass.Bass()
    xg = nc.declare_dram_parameter("xg", [KK, NG, 128], F16, isOutput=False)
    xc64 = nc.declare_dram_parameter("xc64", [KK, NG, 32], F16, isOutput=False)
    wg = nc.declare_dram_parameter("wg", [NG, KK, CE], F16, isOutput=False)
    sel64 = nc.declare_dram_parameter("sel64", [128, 32], F16, isOutput=False)
    sel1 = nc.declare_dram_parameter("sel1", [128, 32], F16, isOutput=False)
    vout = nc.declare_dram_parameter("vout", [BT, CC, EE], F32, isOutput=True)

    with ExitStack() as ctx:
        tc = ctx.enter_context(tile.TileContext(nc))
        singles = ctx.enter_context(tc.tile_pool(name="singles", bufs=1))
        wpool = ctx.enter_context(tc.tile_pool(name="wpool", bufs=3))
        upool = ctx.enter_context(tc.tile_pool(name="upool", bufs=4))
        tpool = ctx.enter_context(tc.tile_pool(name="tpool", bufs=2))
        trpool = ctx.enter_context(tc.tile_pool(name="trpool", bufs=2))
        smpool = ctx.enter_context(tc.tile_pool(name="smpool", bufs=3))
        vpool = ctx.enter_context(tc.tile_pool(name="vpool", bufs=1))
        psum_u = ctx.enter_context(tc.tile_pool(name="psum_u", bufs=2, space="PSUM"))
        psum_s = ctx.enter_context(tc.tile_pool(name="psum_s", bufs=1, space="PSUM"))
        dram = ctx.enter_context(tc.tile_pool(name="dram", bufs=1, space="DRAM"))

        xg_sb = singles.tile([KK, NG, 128], F16)
        nc.sync.dma_start(out=xg_sb[:], in_=xg[:])
        xc64_sb = singles.tile([KK, NG, 32], F16)
        nc.sync.dma_start(out=xc64_sb[:], in_=xc64[:])
        sel64_sb = singles.tile([128, 32], F16)
        nc.sync.dma_start(out=sel64_sb[:], in_=sel64[:])
        sel1_sb = singles.tile([128, 32], F16)
        nc.sync.dma_start(out=sel1_sb[:], in_=sel1[:])

        bB = singles.tile([128, NG, CC], F32)       # logits b after pass B
        vrep = [singles.tile([128, CE], F16, name="vrep0", tag="vrep0"),
                singles.tile([128, CE], F16, name="vrep1", tag="vrep1")]

        def s_to_v(s_ps, it):
            """Evacuate the s psum, AllReduce across cores, squash -> v.
            Builds vrep[it] (fp16, partition-replicated x4); for the last
            iteration writes vout instead."""
            s_sb = vpool.tile([32, CE], F16, tag="s_sb")
            nc.vector.tensor_copy(s_sb[:], s_ps[:])
            sloc = dram.tile([32, CE], F16, tag=f"sloc{it}")
            nc.sync.dma_start(out=sloc[:], in_=s_sb[:])
            ssum = dram.tile([32, CE], F16, tag=f"ssum{it}")
            nc.gpsimd.collective_compute(
                "AllReduce", OP.add,
                replica_groups=[list(range(N_CORES))],
                ins=[sloc[:].opt()], outs=[ssum[:].opt()])
            ssb = vpool.tile([32, CE], F16, tag="ssb")
            nc.sync.dma_start(out=ssb[:], in_=ssum[:])

            # squash scale = ns/(1+ns)/sqrt(ns+eps), ns = sum_e s^2  [32, C]
            s2 = vpool.tile([32, CE], F16, tag="s2")
            nc.vector.tensor_mul(s2[:], ssb[:], ssb[:])
            s2v = s2[:].rearrange("p (e c) -> p c e", e=EE)
            ns = smpool.tile([32, CC], F32, tag="ns")
            nc.vector.tensor_reduce(ns[:], s2v, axis=mybir.AxisListType.X, op=OP.add)
            sq = smpool.tile([32, CC], F32, tag="sq")
            epst = smpool.tile([32, 1], F32, tag="epst")
            nc.vector.memset(epst[:], EPS)
            nc.scalar.activation(sq[:], ns[:], AF.Sqrt, bias=epst[:], scale=1.0)
            den = smpool.tile([32, CC], F32, tag="den")
            nc.vector.scalar_tensor_tensor(den[:], ns[:], 1.0, sq[:],
                                           op0=OP.add, op1=OP.mult)
            inv = smpool.tile([32, CC], F32, tag="inv")
            nc.vector.reciprocal(inv[:], den[:])
            scale = smpool.tile([32, CC], F32, tag="scale")
            nc.vector.tensor_mul(scale[:], ns[:], inv[:])

            if it == 2:
                # v = s*scale, written through a transposed AP so the DMA-out
                # sees contiguous [b, c, e]
                vcm = vpool.tile([32, CE], F32, tag="vcm")
                vcm_t = bass.AP(
                    tensor=vcm[:].tensor, offset=vcm[:].offset,
                    ap=[list(vcm[:].ap[0]), [1, EE], [EE, CC]])
                nc.vector.tensor_mul(vcm_t, ssb[:], _bcast(scale[:], EE, 0))
                nc.sync.dma_start(out=vout[:], in_=vcm[:].rearrange("p (c e) -> p c e", c=CC))
                return

            # replicate s and scale across the 4 partition groups via DRAM
            scd = dram.tile([32, CC], F32, tag=f"scd{it}")
            nc.sync.dma_start(out=scd[:], in_=scale[:])
            screp = smpool.tile([128, CC], F32, tag="screp")
            scd_ap = scd[:]
            rep_sc = bass.AP(tensor=scd_ap.tensor, offset=scd_ap.offset,
                             ap=[[0, 4]] + [list(d) for d in scd_ap.ap])
            nc.sync.dma_start(out=screp[:], in_=rep_sc)
            srep = vpool.tile([128, CE], F16, tag="srep")
            for q in range(2):
                half = ssum[:, q * 1024:(q + 1) * 1024]
                rep_s = bass.AP(tensor=half.tensor, offset=half.offset,
                                ap=[[0, 4]] + [list(d) for d in half.ap])
                nc.sync.dma_start(out=srep[:, q * 1024:(q + 1) * 1024], in_=rep_s)
            nc.vector.tensor_mul(vrep[it][:], srep[:], _bcast(screp[:], EE, 0))

        # ---------------- pass A: s0 = sum_n (u+B) / 64 directly from W ------
        sA = psum_s.tile([32, CE], F32, tag="s_ps")
        for g in range(NG):
            wt = wpool.tile([KK, CE], F16, tag="wt")
            for q in range(4):
                nc.sync.dma_start(out=wt[:, q * 512:(q + 1) * 512],
                                  in_=wg[g, :, q * 512:(q + 1) * 512])
            for q in range(4):
                nc.tensor.matmul(
                    sA[:, q * 512:(q + 1) * 512],
                    xc64_sb[:, g, :],
                    wt[:, q * 512:(q + 1) * 512],
                    start=(g == 0), stop=(g == NG - 1))
        s_to_v(sA, 0)

        # ---------------- passes B (it=1) and C (it=2) -----------------------
        # u is recomputed on the fly (PE, 8-matmul bursts per group pair keep
        # the HAM clock-gate open) instead of streamed from DRAM; psum
        # evacuation rides on ScalarE; all big DVE ops cover a group PAIR to
        # amortize the per-op pipeline overhead; sel-matmuls are emitted one
        # pair late so the PE FIFO never blocks on the current pair's DVE.
        for it in (1, 2):
            sP = psum_s.tile([32, CE], F32, tag="s_ps")
            vr = vrep[it - 1]
            t3_q = []

            def flush_t3(t3p, first, last):
                # start/stop are per psum bank (q-slice)
                for gg in range(2):
                    for q in range(4):
                        nc.tensor.matmul(
                            sP[:, q * 512:(q + 1) * 512],
                            sel1_sb[:],
                            t3p[:, gg, q * 512:(q + 1) * 512],
                            start=(first and gg == 0),
                            stop=(last and gg == 1))

            for gp in range(NG // 2):
                wt = wpool.tile([KK, 2, CE], F16, tag="wt2")
                for gg in range(2):
                    for q in range(2):
                        lo = q * 1024
                        nc.sync.dma_start(
                            out=wt[:, gg, lo:lo + 1024],
                            in_=wg[2 * gp + gg, :, lo:lo + 1024])
                u2 = upool.tile([128, 2, CE], F16, tag="u_full")
                for gg in range(2):
                    for h in range(2):
                        ups = psum_u.tile([128, 1024], F32, tag="ups")
                        for q in range(2):
                            nc.tensor.matmul(
                                ups[:, q * 512:(q + 1) * 512],
                                xg_sb[:, 2 * gp + gg, :],
                                wt[:, gg, h * 1024 + q * 512:
                                   h * 1024 + (q + 1) * 512],
                                start=True, stop=True)
                        nc.scalar.copy(u2[:, gg, h * 1024:(h + 1) * 1024],
                                       ups[:])
                # db = sum_e u*v : fp16 mult + fp16 tree over e (e-major)
                t1 = tpool.tile([128, 2, CE], F16, tag="t1")
                nc.vector.tensor_mul(t1[:], u2[:], _bcast(vr[:], 2, 0))
                t1v = t1[:].rearrange("p g (e c) -> p g e c", e=EE)
                r1 = trpool.tile([128, 2, 16, CC], F16, tag="r1")
                nc.vector.tensor_add(r1[:], t1v[:, :, 0:16, :], t1v[:, :, 16:32, :])
                r2 = trpool.tile([128, 2, 8, CC], F16, tag="r2")
                nc.vector.tensor_add(r2[:], r1[:, :, 0:8, :], r1[:, :, 8:16, :])
                r3 = trpool.tile([128, 2, 4, CC], F16, tag="r3")
                nc.vector.tensor_add(r3[:], r2[:, :, 0:4, :], r2[:, :, 4:8, :])
                r4 = trpool.tile([128, 2, 2, CC], F16, tag="r4")
                nc.vector.tensor_add(r4[:], r3[:, :, 0:2, :], r3[:, :, 2:4, :])
                if it == 1:
                    blog = bB[:, 2 * gp:2 * gp + 2, :]
                    nc.vector.tensor_add(blog, r4[:, :, 0, :], r4[:, :, 1, :])
                else:
                    bt = smpool.tile([128, 2, CC], F32, tag="bt")
                    nc.vector.tensor_add(bt[:], r4[:, :, 0, :], r4[:, :, 1, :])
                    bt2 = smpool.tile([128, 2, CC], F32, tag="bt2")
                    nc.vector.tensor_add(bt2[:], bt[:], bB[:, 2 * gp:2 * gp + 2, :])
                    blog = bt2[:]
                # softmax over c (free axis); Z must stay per-group
                cc = smpool.tile([128, 2, CC], F16, tag="cc")
                for gg in range(2):
                    eb = smpool.tile([128, CC], F32, tag="eb")
                    zz = smpool.tile([128, 1], F32, tag="zz")
                    nc.scalar.activation(eb[:], blog[:, gg, :], AF.Exp,
                                         accum_out=zz[:])
                    iz = smpool.tile([128, 1], F32, tag="iz")
                    nc.vector.reciprocal(iz[:], zz[:])
                    nc.vector.tensor_scalar_mul(cc[:, gg, :], eb[:], iz[:])
                # s += sum_n c*u
                t3 = tpool.tile([128, 2, CE], F16, tag="t3")
                cc_ap = cc[:]
                cc_b = bass.AP(tensor=cc_ap.tensor, offset=cc_ap.offset,
                               ap=[list(cc_ap.ap[0]), list(cc_ap.ap[1]),
                                   [0, EE], list(cc_ap.ap[2])])
                nc.vector.tensor_mul(t3[:], u2[:], cc_b)
                t3_q.append(t3)
                if len(t3_q) > 1:
                    flush_t3(t3_q.pop(0), first=(gp == 1), last=False)
            flush_t3(t3_q.pop(0), first=False, last=True)
            s_to_v(sP, it)

    _split_waits(nc)
    return nc


_CACHE = {}


def _prep_inputs(x, W, B):
    """Host-side layout prep: fp16 casts, n-sharding, block-diagonal x tiles
    (with a ones row for the bias), W permuted to rows=(j,d) cols=(e,c)."""
    x = np.asarray(x, np.float32)
    W = np.asarray(W, np.float32)
    Bmat = np.asarray(B, np.float32)

    # xg[core, k=(j*16+d | 64), nG, m=(j*32+b)]
    xg = np.zeros((N_CORES, KK, NG, 128), np.float16)
    xr = x.transpose(1, 2, 0).reshape(N_CORES, NG, G4, DD, BT)  # [core,g,j,d,b]
    for j in range(G4):
        xg[:, j * DD:(j + 1) * DD, :, j * BT:(j + 1) * BT] = \
            xr[:, :, j].transpose(0, 2, 1, 3)
    xg[:, G4 * DD, :, :] = 1.0

    # collapsed 1/64-scaled x for the direct s0 matmul: dense columns (M=b),
    # rows = all (j, d) pairs; bias row 4/64 (4 n-rows per group, each +B)
    xc64 = np.zeros((N_CORES, KK, NG, 32), np.float16)
    xc64[:, :G4 * DD] = (xr.transpose(0, 2, 3, 1, 4) / NG
                         ).reshape(N_CORES, G4 * DD, NG, BT).astype(np.float16)
    xc64[:, G4 * DD] = G4 / NG

    # wg[core, g, k, e*64+c]
    wg = np.zeros((N_CORES, NG, KK, CE), np.float16)
    Wr = W.reshape(N_CORES, NG, G4, CC, DD, EE).transpose(0, 1, 2, 4, 5, 3)
    wg[:, :, :G4 * DD, :] = Wr.reshape(N_CORES, NG, G4 * DD, CE)
    wg[:, :, G4 * DD, :] = Bmat.T.reshape(CE).astype(np.float16)

    sel64 = np.zeros((128, 32), np.float16)
    sel1 = np.zeros((128, 32), np.float16)
    for p in range(128):
        sel64[p, p % 32] = 1.0 / NG
        sel1[p, p % 32] = 1.0
    return xg, xc64, wg, sel64, sel1


def kernel(x, W, B):
    xg, xc64, wg, sel64, sel1 = _prep_inputs(x, W, B)
    if "nc" not in _CACHE:
        _CACHE["nc"] = _build_program()
    nc = _CACHE["nc"]
    in_maps = [
        {"xg": np.ascontiguousarray(xg[k]),
         "xc64": np.ascontiguousarray(xc64[k]),
         "wg": np.ascontiguousarray(wg[k]),
         "sel64": sel64, "sel1": sel1}
        for k in range(N_CORES)
    ]
    res = run_bass_kernel_spmd(nc, in_maps, list(range(N_CORES)))
    return np.asarray(res.results[0]["vout"], np.float32)


# revision 21
# speedup vs baseline: 1.1094x; 1.1094x over previous
"""CapsuleLayer (dynamic routing) Trainium2 kernel, 8-core SPMD.

Sharding: n_in (2048) split 8 ways -> 256 rows per core. W/x are sharded by n;
the only cross-core data is the [b, c, e] routing sum `s`, AllReduced once per
routing iteration (3x 256KB).

Device layout (per core):
  u[b, n, c, e] is produced by PE matmuls with K = (4n x 16d [+1 bias row]),
  M = 128 = (4 n-offsets x 32 batch), N = (e,c) chunks of 512. A partition row
  p of every on-chip [128, *] tensor is (j, b) = (p // 32, p % 32), i.e. n-local
  offset j within the 4-n group and batch b. The free axis of u is e-major
  (e*64 + c), which keeps every broadcast multiply at DVE 2x mode.

  Routing per iteration (streaming u from a DRAM fp16 scratch):
    db[p, c] = sum_e u * v   -> DVE mult + fp16 tree reduction over e
    softmax over c (free axis) -> ACT Exp with fused accum_out=Z, 1/Z on DVE
    s[b, (e,c)] += sum_n c * u -> DVE mult + PE matmul with a constant 0/1
      selector lhsT (sel[p, m] = p%32==m) contracting the partition axis per-b.
"""

import numpy as np
from contextlib import ExitStack

import concourse.bass as bass
import concourse.tile as tile
from concourse import mybir
from concourse.bass_utils import run_bass_kernel_spmd

F16 = mybir.dt.float16
F32 = mybir.dt.float32
AF = mybir.ActivationFunctionType
OP = mybir.AluOpType

N_CORES = 8
BT, NN, DD = 32, 2048, 16      # batch, n_in, d_in
CC, EE = 64, 32                # n_capsule, d_capsule
NL = NN // N_CORES             # 256 local n rows
G4 = 4                         # n rows per matmul group
NG = NL // G4                  # 64 groups
KK = G4 * DD + 1               # 65 contraction rows (incl. bias row)
CE = CC * EE                   # 2048, stored e-major: col = e*CC + c
EPS = 1e-9


def _split_waits(nc):
    """walrus CTRL codegen only supports one sem-wait per instruction; hoist
    extra waits into preceding NoOps on the same engine."""
    for f in nc.m.functions:
        for bb in f.blocks:
            new_insts = []
            for inst in bb.instructions:
                si = inst.sync_info
                if si is not None and si.on_wait and len(si.on_wait) > 1:
                    waits = list(si.on_wait)
                    for w in waits[:-1]:
                        new_insts.append(mybir.InstNoOp(
                            name=f"WS-{nc.next_id()}",
                            sync_info=mybir.SyncInfo(on_wait=[w], on_update=[]),
                            bass_nofuse=True,
                            engine=inst.engine,
                        ))
                    inst.sync_info = mybir.SyncInfo(
                        on_wait=waits[-1:], on_update=si.on_update)
                new_insts.append(inst)
            bb.instructions = new_insts


def _bcast(ap, n, axis_pos):
    """Insert a [step=0, count=n] dim into an AP at free-dim position axis_pos
    (0 = right after the partition dim)."""
    dims = [list(d) for d in ap.ap]
    dims.insert(1 + axis_pos, [0, n])
    return bass.AP(tensor=ap.tensor, offset=ap.offset, ap=dims)


def _build_program():
    nc = bass.Bass()
    xg = nc.declare_dram_parameter("xg", [KK, NG, 128], F16, isOutput=False)
    xc64 = nc.declare_dram_parameter("xc64", [KK, NG, 32], F16, isOutput=False)
    wg = nc.declare_dram_parameter("wg", [NG, KK, CE], F16, isOutput=False)
    sel64 = nc.declare_dram_parameter("sel64", [128, 32], F16, isOutput=False)
    sel1 = nc.declare_dram_parameter("sel1", [128, 32], F16, isOutput=False)
    vout = nc.declare_dram_parameter("vout", [BT, CC, EE], F32, isOutput=True)

    with ExitStack() as ctx:
        tc = ctx.enter_context(tile.TileContext(nc))
        singles = ctx.enter_context(tc.tile_pool(name="singles", bufs=1))
        wpool = ctx.enter_context(tc.tile_pool(name="wpool", bufs=3))
        upool = ctx.enter_context(tc.tile_pool(name="upool", bufs=5))
        tpool = ctx.enter_context(tc.tile_pool(name="tpool", bufs=2))
        trpool = ctx.enter_context(tc.tile_pool(name="trpool", bufs=2))
        smpool = ctx.enter_context(tc.tile_pool(name="smpool", bufs=3))
        vpool = ctx.enter_context(tc.tile_pool(name="vpool", bufs=1))
        psum_u = ctx.enter_context(tc.tile_pool(name="psum_u", bufs=2, space="PSUM"))
        psum_s = ctx.enter_context(tc.tile_pool(name="psum_s", bufs=1, space="PSUM"))
        dram = ctx.enter_context(tc.tile_pool(name="dram", bufs=1, space="DRAM"))

        xg_sb = singles.tile([KK, NG, 128], F16)
        nc.sync.dma_start(out=xg_sb[:], in_=xg[:])
        xc64_sb = singles.tile([KK, NG, 32], F16)
        nc.sync.dma_start(out=xc64_sb[:], in_=xc64[:])
        sel64_sb = singles.tile([128, 32], F16)
        nc.sync.dma_start(out=sel64_sb[:], in_=sel64[:])
        sel1_sb = singles.tile([128, 32], F16)
        nc.sync.dma_start(out=sel1_sb[:], in_=sel1[:])

        bB = singles.tile([128, NG, CC], F32)       # logits b after pass B
        vrep = [singles.tile([128, CE], F16, name="vrep0", tag="vrep0"),
                singles.tile([128, CE], F16, name="vrep1", tag="vrep1")]

        def s_to_v(s_ps, it):
            """Evacuate the s psum, AllReduce across cores, squash -> v.
            Builds vrep[it] (fp16, partition-replicated x4); for the last
            iteration writes vout instead."""
            s_sb = vpool.tile([32, CE], F16, tag="s_sb")
            nc.vector.tensor_copy(s_sb[:], s_ps[:])
            sloc = dram.tile([32, CE], F16, tag=f"sloc{it}")
            nc.sync.dma_start(out=sloc[:], in_=s_sb[:])
            ssum = dram.tile([32, CE], F16, tag=f"ssum{it}")
            nc.gpsimd.collective_compute(
                "AllReduce", OP.add,
                replica_groups=[list(range(N_CORES))],
                ins=[sloc[:].opt()], outs=[ssum[:].opt()])
            ssb = vpool.tile([32, CE], F16, tag="ssb")
            nc.sync.dma_start(out=ssb[:], in_=ssum[:])

            # squash scale = ns/(1+ns)/sqrt(ns+eps), ns = sum_e s^2  [32, C]
            s2 = vpool.tile([32, CE], F16, tag="s2")
            nc.vector.tensor_mul(s2[:], ssb[:], ssb[:])
            s2v = s2[:].rearrange("p (e c) -> p c e", e=EE)
            ns = smpool.tile([32, CC], F32, tag="ns")
            nc.vector.tensor_reduce(ns[:], s2v, axis=mybir.AxisListType.X, op=OP.add)
            sq = smpool.tile([32, CC], F32, tag="sq")
            epst = smpool.tile([32, 1], F32, tag="epst")
            nc.vector.memset(epst[:], EPS)
            nc.scalar.activation(sq[:], ns[:], AF.Sqrt, bias=epst[:], scale=1.0)
            den = smpool.tile([32, CC], F32, tag="den")
            nc.vector.scalar_tensor_tensor(den[:], ns[:], 1.0, sq[:],
                                           op0=OP.add, op1=OP.mult)
            inv = smpool.tile([32, CC], F32, tag="inv")
            nc.vector.reciprocal(inv[:], den[:])
            scale = smpool.tile([32, CC], F32, tag="scale")
            nc.vector.tensor_mul(scale[:], ns[:], inv[:])

            if it == 2:
                # v = s*scale, written through a transposed AP so the DMA-out
                # sees contiguous [b, c, e]
                vcm = vpool.tile([32, CE], F32, tag="vcm")
                vcm_t = bass.AP(
                    tensor=vcm[:].tensor, offset=vcm[:].offset,
                    ap=[list(vcm[:].ap[0]), [1, EE], [EE, CC]])
                nc.vector.tensor_mul(vcm_t, ssb[:], _bcast(scale[:], EE, 0))
                nc.sync.dma_start(out=vout[:], in_=vcm[:].rearrange("p (c e) -> p c e", c=CC))
                return

            # replicate s and scale across the 4 partition groups via DRAM
            scd = dram.tile([32, CC], F32, tag=f"scd{it}")
            nc.sync.dma_start(out=scd[:], in_=scale[:])
            screp = smpool.tile([128, CC], F32, tag="screp")
            scd_ap = scd[:]
            rep_sc = bass.AP(tensor=scd_ap.tensor, offset=scd_ap.offset,
                             ap=[[0, 4]] + [list(d) for d in scd_ap.ap])
            nc.sync.dma_start(out=screp[:], in_=rep_sc)
            srep = vpool.tile([128, CE], F16, tag="srep")
            for q in range(2):
                half = ssum[:, q * 1024:(q + 1) * 1024]
                rep_s = bass.AP(tensor=half.tensor, offset=half.offset,
                                ap=[[0, 4]] + [list(d) for d in half.ap])
                nc.sync.dma_start(out=srep[:, q * 1024:(q + 1) * 1024], in_=rep_s)
            nc.vector.tensor_mul(vrep[it][:], srep[:], _bcast(screp[:], EE, 0))

        # ---------------- pass A: s0 = sum_n (u+B) / 64 directly from W ------
        sA = psum_s.tile([32, CE], F32, tag="s_ps")
        for gp in range(NG // 2):
            wt = wpool.tile([KK, 2, CE], F16, tag="wt2")
            for gg in range(2):
                eng = nc.sync if gg == 0 else nc.gpsimd
                for q in range(2):
                    lo = q * 1024
                    eng.dma_start(out=wt[:, gg, lo:lo + 1024],
                                  in_=wg[2 * gp + gg, :, lo:lo + 1024])
            for gg in range(2):
                for q in range(4):
                    nc.tensor.matmul(
                        sA[:, q * 512:(q + 1) * 512],
                        xc64_sb[:, 2 * gp + gg, :],
                        wt[:, gg, q * 512:(q + 1) * 512],
                        start=(gp == 0 and gg == 0),
                        stop=(gp == NG // 2 - 1 and gg == 1))
        s_to_v(sA, 0)

        # ---------------- passes B (it=1) and C (it=2) -----------------------
        # u is recomputed on the fly (PE, 8-matmul bursts per group pair keep
        # the HAM clock-gate open) instead of streamed from DRAM; psum
        # evacuation rides on ScalarE; all big DVE ops cover a group PAIR to
        # amortize the per-op pipeline overhead; sel-matmuls are emitted one
        # pair late so the PE FIFO never blocks on the current pair's DVE.
        for it in (1, 2):
            sP = psum_s.tile([32, CE], F32, tag="s_ps")
            vr = vrep[it - 1]
            t3_q = []

            def flush_t3(t3p, first, last):
                # start/stop are per psum bank (q-slice)
                for gg in range(2):
                    for q in range(4):
                        nc.tensor.matmul(
                            sP[:, q * 512:(q + 1) * 512],
                            sel1_sb[:],
                            t3p[:, gg, q * 512:(q + 1) * 512],
                            start=(first and gg == 0),
                            stop=(last and gg == 1))

            for gp in range(NG // 2):
                wt = wpool.tile([KK, 2, CE], F16, tag="wt2")
                for gg in range(2):
                    eng = nc.sync if gg == 0 else nc.gpsimd
                    for q in range(2):
                        lo = q * 1024
                        eng.dma_start(
                            out=wt[:, gg, lo:lo + 1024],
                            in_=wg[2 * gp + gg, :, lo:lo + 1024])
                u2 = upool.tile([128, 2, CE], F16, tag="u_full")
                for gg in range(2):
                    for h in range(2):
                        ups = psum_u.tile([128, 1024], F32, tag="ups")
                        for q in range(2):
                            nc.tensor.matmul(
                                ups[:, q * 512:(q + 1) * 512],
                                xg_sb[:, 2 * gp + gg, :],
                                wt[:, gg, h * 1024 + q * 512:
                                   h * 1024 + (q + 1) * 512],
                                start=True, stop=True)
                        nc.scalar.copy(u2[:, gg, h * 1024:(h + 1) * 1024],
                                       ups[:])
                # db = sum_e u*v : fp16 mult + fp16 tree over e (e-major)
                t1 = tpool.tile([128, 2, CE], F16, tag="t1")
                nc.vector.tensor_mul(t1[:], u2[:], _bcast(vr[:], 2, 0))
                t1v = t1[:].rearrange("p g (e c) -> p g e c", e=EE)
                r1 = trpool.tile([128, 2, 16, CC], F16, tag="r1")
                nc.vector.tensor_add(r1[:], t1v[:, :, 0:16, :], t1v[:, :, 16:32, :])
                r2 = trpool.tile([128, 2, 8, CC], F16, tag="r2")
                nc.vector.tensor_add(r2[:], r1[:, :, 0:8, :], r1[:, :, 8:16, :])
                r3 = trpool.tile([128, 2, 4, CC], F16, tag="r3")
                nc.vector.tensor_add(r3[:], r2[:, :, 0:4, :], r2[:, :, 4:8, :])
                r4 = trpool.tile([128, 2, 2, CC], F16, tag="r4")
                nc.vector.tensor_add(r4[:], r3[:, :, 0:2, :], r3[:, :, 2:4, :])
                if it == 1:
                    blog = bB[:, 2 * gp:2 * gp + 2, :]
                    nc.vector.tensor_add(blog, r4[:, :, 0, :], r4[:, :, 1, :])
                else:
                    bt = smpool.tile([128, 2, CC], F32, tag="bt")
                    nc.vector.tensor_add(bt[:], r4[:, :, 0, :], r4[:, :, 1, :])
                    bt2 = smpool.tile([128, 2, CC], F32, tag="bt2")
                    nc.vector.tensor_add(bt2[:], bt[:], bB[:, 2 * gp:2 * gp + 2, :])
                    blog = bt2[:]
                # softmax over c (free axis); Z must stay per-group
                cc = smpool.tile([128, 2, CC], F16, tag="cc")
                for gg in range(2):
                    eb = smpool.tile([128, CC], F32, tag="eb")
                    zz = smpool.tile([128, 1], F32, tag="zz")
                    nc.scalar.activation(eb[:], blog[:, gg, :], AF.Exp,
                                         accum_out=zz[:])
                    iz = smpool.tile([128, 1], F32, tag="iz")
                    nc.vector.reciprocal(iz[:], zz[:])
                    nc.vector.tensor_scalar_mul(cc[:, gg, :], eb[:], iz[:])
                # s += sum_n c*u
                t3 = tpool.tile([128, 2, CE], F16, tag="t3")
                cc_ap = cc[:]
                cc_b = bass.AP(tensor=cc_ap.tensor, offset=cc_ap.offset,
                               ap=[list(cc_ap.ap[0]), list(cc_ap.ap[1]),
                                   [0, EE], list(cc_ap.ap[2])])
                nc.vector.tensor_mul(t3[:], u2[:], cc_b)
                t3_q.append(t3)
                if len(t3_q) > 1:
                    flush_t3(t3_q.pop(0), first=(gp == 1), last=False)
            flush_t3(t3_q.pop(0), first=False, last=True)
            s_to_v(sP, it)

    _split_waits(nc)
    return nc


_CACHE = {}


def _prep_inputs(x, W, B):
    """Host-side layout prep: fp16 casts, n-sharding, block-diagonal x tiles
    (with a ones row for the bias), W permuted to rows=(j,d) cols=(e,c)."""
    x = np.asarray(x, np.float32)
    W = np.asarray(W, np.float32)
    Bmat = np.asarray(B, np.float32)

    # xg[core, k=(j*16+d | 64), nG, m=(j*32+b)]
    xg = np.zeros((N_CORES, KK, NG, 128), np.float16)
    xr = x.transpose(1, 2, 0).reshape(N_CORES, NG, G4, DD, BT)  # [core,g,j,d,b]
    for j in range(G4):
        xg[:, j * DD:(j + 1) * DD, :, j * BT:(j + 1) * BT] = \
            xr[:, :, j].transpose(0, 2, 1, 3)
    xg[:, G4 * DD, :, :] = 1.0

    # collapsed 1/64-scaled x for the direct s0 matmul: dense columns (M=b),
    # rows = all (j, d) pairs; bias row 4/64 (4 n-rows per group, each +B)
    xc64 = np.zeros((N_CORES, KK, NG, 32), np.float16)
    xc64[:, :G4 * DD] = (xr.transpose(0, 2, 3, 1, 4) / NG
                         ).reshape(N_CORES, G4 * DD, NG, BT).astype(np.float16)
    xc64[:, G4 * DD] = G4 / NG

    # wg[core, g, k, e*64+c]
    wg = np.zeros((N_CORES, NG, KK, CE), np.float16)
    Wr = W.reshape(N_CORES, NG, G4, CC, DD, EE).transpose(0, 1, 2, 4, 5, 3)
    wg[:, :, :G4 * DD, :] = Wr.reshape(N_CORES, NG, G4 * DD, CE)
    wg[:, :, G4 * DD, :] = Bmat.T.reshape(CE).astype(np.float16)

    sel64 = np.zeros((128, 32), np.float16)
    sel1 = np.zeros((128, 32), np.float16)
    for p in range(128):
        sel64[p, p % 32] = 1.0 / NG
        sel1[p, p % 32] = 1.0
    return xg, xc64, wg, sel64, sel1


def kernel(x, W, B):
    xg, xc64, wg, sel64, sel1 = _prep_inputs(x, W, B)
    if "nc" not in _CACHE:
        _CACHE["nc"] = _build_program()
    nc = _CACHE["nc"]
    in_maps = [
        {"xg": np.ascontiguousarray(xg[k]),
         "xc64": np.ascontiguousarray(xc64[k]),
         "wg": np.ascontiguousarray(wg[k]),
         "sel64": sel64, "sel1": sel1}
        for k in range(N_CORES)
    ]
    res = run_bass_kernel_spmd(nc, in_maps, list(range(N_CORES)))
    return np.asarray(res.results[0]["vout"], np.float32)
